# revision 28
# baseline (speedup 1.0000x reference)
"""Sliding-window GQA attention (Gemma-style) on 8 Trainium2 NeuronCores.

Sharding: data-parallel over tokens. B=2, T=2048 -> 4096 tokens -> 512
queries per core (core c = 4*b + j handles batch b, queries [512j, 512j+512)).
Each core recomputes k/v for its fixed local window of 1536 tokens
[qs-1024, qs+512) (zero-padded on the left at sequence start), so all 8 cores
run one identical NEFF; per-core differences live entirely in the input data
(sliced x, RoPE tables, den-correction table).

On-chip dataflow (per core):
  phase 1: q/k/v projections with W stationary and x^T moving -> q^T/k^T/v^T
           [H=128 partitions, tokens]; RMSNorm via ones-matmul column sums +
           on1-matmul broadcast of 1/std; norm scales (and SCALE) folded into
           the RoPE tables host-side; 1/std_k folded INTO kTn so attention
           logits need no per-s rescale. RoPE runs in bf16 (head-dim permuted
           on host so rotate-half is a quadrant-local stream_shuffle). v^T
           transposed back to [s, h] via PE transposes.
  phase 2: logits = kTn-tile.T @ qTn-pair (both heads of a kv group, N=256)
           into paired psum; exp on ACT (wide, scale-free since k is
           pre-normalized); boundary s-tiles masked by constant triangles;
           sliding-window validity handled by a host den-correction table
           (invalid tiles contribute exp(0)=1, subtracted from the den).
           PV with ones-column vsb gives enc and den in one accumulation.
  phase 3: output projection accumulating over heads.
"""

import numpy as np
import ml_dtypes

import concourse.bass as bass
import concourse.mybir as mybir
import concourse.tile as tile
from concourse import library_config
from concourse.masks import make_identity
from concourse.bass_utils import run_bass_kernel_spmd

AF = mybir.ActivationFunctionType
ALU = mybir.AluOpType
F32 = mybir.dt.float32
BF16 = mybir.dt.bfloat16

B, T, D = 2, 2048, 2048
N, K, H = 16, 8, 128
G = N // K
SOFT_CAP = 50.0
WINDOW = 1024
SCALE = H ** -0.5
ROPE_BASE = 10000.0
EPS = 1e-6

TQ = 512            # queries per core
TKV = 1536          # kv window per core
VST = 129           # per-s-tile width in vsb: 128 v cols + ones column
NQT = TQ // 128     # 4 q-tiles
NST = TKV // 128    # 12 s-tiles
ND = D // 128       # 16 d-tiles
NWIN = 9            # s-tiles in a q-tile's window
NCORES = 8
EVW = 132           # per-g stride in the ev psum tile (16B aligned)

# quadrant-local half swap for stream_shuffle (32-partition groups)
SWAP16 = list(range(16, 32)) + list(range(16))


def _rope_perm():
    """orig[p] = original head-dim index stored at partition p; freq[p];
    sign[p] for the sin table."""
    orig = np.zeros(128, np.int64)
    freq = np.zeros(128, np.int64)
    sign = np.zeros(128, np.float32)
    for p in range(128):
        qd, o = divmod(p, 32)
        if o < 16:
            orig[p] = 16 * qd + o
            freq[p] = 16 * qd + o
            sign[p] = -1.0
        else:
            orig[p] = 64 + 16 * qd + (o - 16)
            freq[p] = 16 * qd + (o - 16)
            sign[p] = 1.0
    return orig, freq, sign


_ORIG, _FREQ, _SIGN = _rope_perm()
# partner index under SWAP16 (the other half of each rotate pair)
_PARTNER = np.array([32 * (p // 32) + (p % 32 + 16) % 32 for p in range(128)])

_module_cache = {}

# Instruction types lowered to CTRL encodings: the walrus build in this
# container supports only ONE sync-wait on them ("Too many sync wait
# commands" / "ISA wrong length" in codegen otherwise).
_CTRL_TYPES = ("InstDrain", "InstNoOp", "InstISA", "InstEventSemaphore")


def _split_ctrl_multiwaits(nc, maxw=1):
    """Move excess sem-waits off CTRL-type instructions onto preceding
    same-engine NoOps (same engine queue => identical ordering semantics)."""
    import concourse.mybir as mybir
    for f in nc.m.functions:
        for blk in f.blocks:
            insts = blk.instructions
            out = []
            changed = False
            for inst in insts:
                si = inst.sync_info
                if (si is not None and si.on_wait
                        and len(si.on_wait) > maxw):
                    waits = list(si.on_wait)
                    extra, keep = waits[:-maxw], waits[-maxw:]
                    for k, w in enumerate(extra):
                        nop = mybir.InstNoOp(name=f"{inst.name}-ws{k}",
                                             ins=[], outs=[])
                        nop.engine = inst.engine
                        nop.sync_info = mybir.SyncInfo(on_wait=[w],
                                                       on_update=[])
                        out.append(nop)
                    si.on_wait = keep
                    changed = True
                out.append(inst)
            if changed:
                insts[:] = out


def _build_module(split=True):
    nc = bass.Bass("TRN2", target_bir_lowering=False, debug=False)

    xt_d = nc.dram_tensor("xt", (D, TKV), BF16, kind="ExternalInput").ap()
    wq_d = nc.dram_tensor("wq", (N, D, H), BF16, kind="ExternalInput").ap()
    wk_d = nc.dram_tensor("wk", (K, D, H), BF16, kind="ExternalInput").ap()
    wv_d = nc.dram_tensor("wv", (ND, 128, K * H), BF16,
                          kind="ExternalInput").ap()
    wo_d = nc.dram_tensor("wo", (N, H, D), BF16, kind="ExternalInput").ap()
    ckq_d = nc.dram_tensor("ckq", (H, TQ), BF16, kind="ExternalInput").ap()
    skq_d = nc.dram_tensor("skq", (H, TQ), BF16, kind="ExternalInput").ap()
    ckk_d = nc.dram_tensor("ckk", (H, TKV), BF16, kind="ExternalInput").ap()
    skk_d = nc.dram_tensor("skk", (H, TKV), BF16, kind="ExternalInput").ap()
    tri_d = nc.dram_tensor("tri", (128, 512), BF16, kind="ExternalInput").ap()
    corr_d = nc.dram_tensor("corr", (128, NQT), F32,
                            kind="ExternalInput").ap()
    idb_d = nc.dram_tensor("idb", (128, 128), BF16, kind="ExternalInput").ap()
    out_d = nc.dram_tensor("out", (TQ, D), F32, kind="ExternalOutput").ap()

    with tile.TileContext(nc) as tc:
        with tc.tile_pool(name="const", bufs=1) as cst, \
             tc.tile_pool(name="acc", bufs=1) as acc, \
             tc.tile_pool(name="wst", bufs=2) as wst, \
             tc.tile_pool(name="wvp", bufs=4) as wvp, \
             tc.tile_pool(name="wost", bufs=5) as wost, \
             tc.tile_pool(name="scr", bufs=2) as scr, \
             tc.tile_pool(name="pp", bufs=3) as pp, \
             tc.tile_pool(name="psA", bufs=4, space="PSUM") as psA, \
             tc.tile_pool(name="psB", bufs=4, space="PSUM") as psB:

            # ---- constants / preloads ----
            # first two q-heads' weights load before the big xts transfer so
            # the first projection matmuls start as soon as x^T tiles land
            w_pre = {}
            for n0 in range(2):
                wp = wst.tile([128, ND * H], BF16, tag="w", name=f"w_pre{n0}")
                nc.sync.dma_start(
                    wp[:].rearrange("p (d h) -> p d h", d=ND),
                    wq_d[n0].rearrange("(d p) h -> p d h", p=128))
                w_pre[n0] = wp
            xts = cst.tile([128, ND * TKV], BF16, tag="xts")
            xt_g = xt_d.rearrange("(g d p) t -> g p d t", g=4, p=128)
            for g4 in range(4):
                nc.sync.dma_start(
                    xts[:, g4 * 4 * TKV:(g4 + 1) * 4 * TKV].rearrange(
                        "p (d t) -> p d t", d=4),
                    xt_g[g4])
            ckq_t = cst.tile([H, TQ], BF16, tag="ckq")
            nc.sync.dma_start(ckq_t[:], ckq_d[:])
            skq_t = cst.tile([H, TQ], BF16, tag="skq")
            nc.sync.dma_start(skq_t[:], skq_d[:])
            ckk_t = cst.tile([H, TKV], BF16, tag="ckk")
            nc.sync.dma_start(ckk_t[:], ckk_d[:])
            skk_t = cst.tile([H, TKV], BF16, tag="skk")
            nc.sync.dma_start(skk_t[:], skk_d[:])
            tri_t = cst.tile([128, 512], BF16, tag="tri")
            nc.sync.dma_start(tri_t[:], tri_d[:])
            corr_t = cst.tile([128, NQT], F32, tag="corr")
            nc.sync.dma_start(corr_t[:], corr_d[:])
            idb_t = cst.tile([128, 128], BF16, tag="idb")
            nc.sync.dma_start(idb_t[:], idb_d[:])
            ones_bf = cst.tile([128, 1], BF16, tag="ones")
            nc.vector.memset(ones_bf[:], 1.0)
            on1 = cst.tile([1, 128], F32, tag="on1")
            nc.vector.memset(on1[:], 1.0)
            eps_t = cst.tile([1, 1], F32, tag="eps")
            nc.vector.memset(eps_t[:], EPS)

            # ---- big accumulators ----
            qTn = acc.tile([128, N * TQ], BF16, tag="qTn")
            kTn = acc.tile([128, K * TKV], BF16, tag="kTn")
            vsb = acc.tile([128, K * NST * VST], BF16, tag="vsb")
            nc.vector.memset(vsb[:], 1.0)
            encT = acc.tile([128, N * NQT * 128], BF16, tag="encT")

            def rope(src_bf, cka, ska, out_slice, strided=False):
                rot = scr.tile([128, 512], BF16, tag="rot")
                nc.vector.stream_shuffle(rot[:], src_bf[:], SWAP16)
                t1 = scr.tile([128, 512], BF16, tag="t1")
                nc.vector.tensor_mul(t1[:], src_bf[:], cka)
                t2 = scr.tile([128, 512], BF16, tag="t2")
                nc.gpsimd.tensor_mul(t2[:], rot[:], ska)
                if strided:
                    nc.vector.tensor_add(
                        out_slice,
                        t1[:].rearrange("p (a b) -> p a b", a=NQT),
                        t2[:].rearrange("p (a b) -> p a b", a=NQT))
                else:
                    nc.vector.tensor_add(out_slice, t1[:], t2[:])

            # ---- phase 1a: q heads ----
            q_s1 = q_s2 = None
            for n in range(N + 2):
                nstate = None
                if n < N:
                    if n in w_pre:
                        w_t = w_pre[n]
                    else:
                        w_t = wst.tile([128, ND * H], BF16, tag="w")
                        nc.sync.dma_start(
                            w_t[:].rearrange("p (d h) -> p d h", d=ND),
                            wq_d[n].rearrange("(d p) h -> p d h", p=128))
                    ps = psA.tile([128, 512], F32, tag="big")
                    for d in range(ND):
                        nc.tensor.matmul(
                            ps[:], w_t[:, d * H:(d + 1) * H],
                            xts[:, d * TKV + 1024:d * TKV + 1536],
                            start=(d == 0), stop=(d == ND - 1))
                    sq = scr.tile([128, 512], BF16, tag="sq")
                    nc.scalar.activation(sq[:], ps[:], AF.Square)
                    nstate = (n, ps, sq)
                if q_s1 is not None:
                    n1, ps1, sq1 = q_s1
                    ssp = psB.tile([1, 512], F32, tag="sm")
                    nc.tensor.matmul(ssp[:], ones_bf[:], sq1[:],
                                     start=True, stop=True)
                    lns = scr.tile([1, 512], F32, tag="row")
                    nc.scalar.activation(lns[:], ssp[:], AF.Ln,
                                         scale=1.0 / H, bias=eps_t[:])
                    rst = scr.tile([1, 512], F32, tag="row2")
                    nc.scalar.activation(rst[:], lns[:], AF.Exp, scale=-0.5)
                    q_s1 = (n1, ps1, rst)
                if q_s2 is not None:
                    n2, ps2, rst2 = q_s2
                    rbp = psB.tile([128, 512], F32, tag="sm")
                    nc.tensor.matmul(rbp[:], on1[:], rst2[:],
                                     start=True, stop=True)
                    rstb = scr.tile([128, 512], BF16, tag="rstb")
                    nc.vector.tensor_copy(rstb[:], rbp[:])
                    qn = scr.tile([128, 512], BF16, tag="qn")
                    nc.vector.tensor_mul(qn[:], ps2[:], rstb[:])
                    # qTn layout: [128, (kh, qi, g, t)] so the two q-heads of
                    # a kv group sit adjacent per q-tile (256-wide logits rhs)
                    kh2, g2 = divmod(n2, G)
                    qfull = qTn[:]
                    q_ap = bass.AP(
                        qfull.tensor,
                        qfull.offset + (kh2 * G * NQT + g2) * 128,
                        [qfull.ap[0], [G * 128, NQT], [1, 128]])
                    rope(qn, ckq_t[:, 0:512], skq_t[:, 0:512], q_ap,
                         strided=True)
                q_s2 = q_s1
                q_s1 = nstate

            # ---- phase 2: attention (interleaved with the k projection) ----
            a_s1 = a_s2 = None
            iters = [(kh, qi) for kh in range(K) for qi in range(NQT)]
            a_it = [0]
            # psum pairing for wide exp: (r0,r8) share a tile (both get the
            # triangle mask), then (1,2),(3,4),(5,6),(7,)
            PAIRS = [(0, 8), (1, 2), (3, 4), (5, 6), (7,)]

            def attn_step(it):
                nstate = None
                if it < len(iters):
                        kh, qi = iters[it]
                        probs = pp.tile([128, NWIN * 256], BF16, tag="probs")
                        eeT = scr.tile([128, 512], BF16, tag="eeT")
                        for pr in PAIRS:
                            lgp = psB.tile([128, 256 * len(pr)], F32,
                                           tag="sm")
                            for i, r in enumerate(pr):
                                st = qi + r
                                nc.tensor.matmul(
                                    lgp[:, i * 256:(i + 1) * 256],
                                    kTn[:, kh * TKV + st * 128:
                                        kh * TKV + (st + 1) * 128],
                                    qTn[:, (kh * NQT + qi) * 256:
                                        (kh * NQT + qi + 1) * 256],
                                    start=True, stop=True)
                            if pr[0] == 0:
                                out = eeT[:]
                            else:
                                out = probs[:, pr[0] * 256:
                                            (pr[-1] + 1) * 256]
                            nc.scalar.activation(out, lgp[:], AF.Exp)
                        # boundary triangle masks for r=0 and r=8
                        pfull = probs[:]
                        p_ap = bass.AP(
                            pfull.tensor, pfull.offset,
                            [pfull.ap[0], [(NWIN - 1) * 256, 2], [1, 256]])
                        nc.vector.tensor_tensor(
                            p_ap,
                            eeT[:].rearrange("p (a b) -> p a b", a=2),
                            tri_t[:].rearrange("p (a b) -> p a b", a=2),
                            op=ALU.mult)
                        nstate = (kh, qi, probs)
                if a_s2 is not None:
                        kh0, qi0, probs0 = a_s2
                        ev = psB.tile([128, 2 * EVW], F32, tag="sm")
                        for g in range(G):
                            for r in range(NWIN):
                                st = qi0 + r
                                off = (kh0 * NST + st) * VST
                                nc.tensor.matmul(
                                    ev[:, g * EVW:g * EVW + VST],
                                    probs0[:, r * 256 + g * 128:
                                           r * 256 + (g + 1) * 128],
                                    vsb[:, off:off + VST],
                                    start=(r == 0), stop=(r == NWIN - 1))
                        evf = ev[:]
                        den_ap = bass.AP(evf.tensor, evf.offset + 128,
                                         [evf.ap[0], [EVW, 2]])
                        den2 = scr.tile([128, 2], F32, tag="den2")
                        nc.vector.tensor_scalar(
                            den2[:], den_ap, corr_t[:, qi0:qi0 + 1], None,
                            op0=ALU.subtract)
                        rden2 = scr.tile([128, 2], F32, tag="rden2")
                        nc.vector.reciprocal(rden2[:], den2[:])
                        v_ap = bass.AP(evf.tensor, evf.offset,
                                       [evf.ap[0], [EVW, 2], [1, 128]])
                        rdf = rden2[:]
                        rd_ap = bass.AP(rdf.tensor, rdf.offset,
                                        [rdf.ap[0], [1, 2], [0, 128]])
                        enc_sb = scr.tile([128, 256], BF16, tag="encsb")
                        nc.vector.tensor_tensor(
                            enc_sb[:].rearrange("p (a b) -> p a b", a=2),
                            v_ap, rd_ap, op=ALU.mult)
                        etp = psB.tile([128, 256], BF16, tag="sm")
                        for g in range(G):
                            nc.tensor.matmul(
                                etp[:, g * 128:(g + 1) * 128],
                                enc_sb[:, g * 128:(g + 1) * 128], idb_t[:],
                                is_transpose=True, start=True, stop=True)
                        efull = encT[:]
                        e_ap = bass.AP(
                            efull.tensor,
                            efull.offset + (kh0 * G * NQT + qi0) * 128,
                            [efull.ap[0], [NQT * 128, 2], [1, 128]])
                        nc.vector.tensor_copy(
                            e_ap, etp[:].rearrange("p (a b) -> p a b", a=2))
                return nstate

            # ---- phase 1c: v projection, directly in [s, h] orientation ----
            # stationary = x^T d-tile [128 d, 128 s]; moving = wv for 4 kv
            # heads [128 d, 512]; 3 rounds x 4 s-tiles x 2 head-halves use
            # all 8 psum banks; wv streams per d-tile (re-read each round).
            for c in range(3):
                vps = []
                for stq in range(4):
                    pa = psA.tile([128, 512], F32, tag="big",
                                  name=f"vA_{c}_{stq}")
                    pb = psB.tile([128, 512], F32, tag="sm",
                                  name=f"vB_{c}_{stq}")
                    vps.append((pa, pb))
                for d in range(ND):
                    wvt = wvp.tile([128, K * H], BF16, tag="wv")
                    nc.sync.dma_start(wvt[:], wv_d[d])
                    for stq in range(4):
                        pa, pb = vps[stq]
                        xsl = xts[:, d * TKV + c * 512 + stq * 128:
                                  d * TKV + c * 512 + (stq + 1) * 128]
                        nc.tensor.matmul(pa[:], xsl, wvt[:, 0:512],
                                         start=(d == 0), stop=(d == ND - 1))
                        nc.tensor.matmul(pb[:], xsl, wvt[:, 512:1024],
                                         start=(d == 0), stop=(d == ND - 1))
                for stq in range(4):
                    st = c * 4 + stq
                    pa, pb = vps[stq]
                    vf = vsb[:]
                    for half, pt in ((0, pa), (1, pb)):
                        v_ap = bass.AP(
                            vf.tensor,
                            vf.offset + (half * 4 * NST + st) * VST,
                            [vf.ap[0], [NST * VST, 4], [1, 128]])
                        nc.vector.tensor_copy(
                            v_ap, pt[:].rearrange("p (a b) -> p a b", a=4))

            def attn_advance(k_steps):
                nonlocal a_s1, a_s2
                for _ in range(k_steps):
                    if a_it[0] >= len(iters) + 2:
                        return
                    ns = attn_step(a_it[0])
                    a_it[0] += 1
                    a_s2 = a_s1
                    a_s1 = ns

            # ---- phase 1b: k heads (1/std_k folded into kTn), attention
            # iters interleaved per completed kv head so the two MM streams
            # fill each other's dependency bubbles ----
            k_s1 = k_s2 = None
            nchunks = K * 3
            for ci in range(nchunks + 2):
                nstate = None
                if ci < nchunks:
                    kh, c = divmod(ci, 3)
                    if c == 0:
                        w_t = wst.tile([128, ND * H], BF16, tag="w")
                        nc.sync.dma_start(
                            w_t[:].rearrange("p (d h) -> p d h", d=ND),
                            wk_d[kh].rearrange("(d p) h -> p d h", p=128))
                    ps = psA.tile([128, 512], F32, tag="big")
                    for d in range(ND):
                        nc.tensor.matmul(
                            ps[:], w_t[:, d * H:(d + 1) * H],
                            xts[:, d * TKV + c * 512:d * TKV + (c + 1) * 512],
                            start=(d == 0), stop=(d == ND - 1))
                    sq = scr.tile([128, 512], BF16, tag="sq")
                    nc.scalar.activation(sq[:], ps[:], AF.Square)
                    nstate = (kh, c, ps, sq)
                if k_s1 is not None:
                    kh1, c1, ps1, sq1 = k_s1
                    ssp = psB.tile([1, 512], F32, tag="sm")
                    nc.tensor.matmul(ssp[:], ones_bf[:], sq1[:],
                                     start=True, stop=True)
                    lns = scr.tile([1, 512], F32, tag="row")
                    nc.scalar.activation(lns[:], ssp[:], AF.Ln,
                                         scale=1.0 / H, bias=eps_t[:])
                    rks = scr.tile([1, 512], F32, tag="row2")
                    nc.scalar.activation(rks[:], lns[:], AF.Exp, scale=-0.5)
                    k_s1 = (kh1, c1, ps1, rks)
                if k_s2 is not None:
                    kh2, c2, ps2, rks2 = k_s2
                    rbp = psB.tile([128, 512], F32, tag="sm")
                    nc.tensor.matmul(rbp[:], on1[:], rks2[:],
                                     start=True, stop=True)
                    rkb = scr.tile([128, 512], BF16, tag="rstb")
                    nc.vector.tensor_copy(rkb[:], rbp[:])
                    kn = scr.tile([128, 512], BF16, tag="qn")
                    nc.vector.tensor_mul(kn[:], ps2[:], rkb[:])
                    rope(kn, ckk_t[:, c2 * 512:(c2 + 1) * 512],
                         skk_t[:, c2 * 512:(c2 + 1) * 512],
                         kTn[:, kh2 * TKV + c2 * 512:
                             kh2 * TKV + (c2 + 1) * 512])
                    # release attention iters at ~4/3 per chunk (lag 2) so
                    # the k and attention MM streams stay finely interleaved;
                    # never past the last completed kv head
                    ci2 = 3 * kh2 + c2
                    avail = NQT * ((ci2 + 1) // 3)
                    target = min(avail, max(0, (NQT * (ci2 + 1)) // 3 - 2))
                    attn_advance(target - a_it[0])
                k_s2 = k_s1
                k_s1 = nstate
            attn_advance(len(iters) + 2 - a_it[0])

            # ---- phase 3: output projection (2 rounds of dc-pairs, all 8
            # psum banks, one wo DMA per (n, round)) ----
            for half in range(2):
                ops = []
                for dcl in range(2):
                    pool = psA if dcl == 0 else psB
                    tg = "big" if dcl == 0 else "sm"
                    ops.append([pool.tile([128, 512], F32, tag=tg,
                                          name=f"op_{half}_{dcl}_{qi}")
                                for qi in range(NQT)])
                for n in range(N):
                    wo_sl = wost.tile([128, 1024], BF16, tag="wo")
                    nc.sync.dma_start(
                        wo_sl[:], wo_d[n][:, half * 1024:(half + 1) * 1024])
                    for dcl in range(2):
                        for qi in range(NQT):
                            nc.tensor.matmul(
                                ops[dcl][qi][:],
                                encT[:, (n * NQT + qi) * 128:
                                     (n * NQT + qi + 1) * 128],
                                wo_sl[:, dcl * 512:(dcl + 1) * 512],
                                start=(n == 0), stop=(n == N - 1))
                for dcl in range(2):
                    dc = half * 2 + dcl
                    for qi in range(NQT):
                        osb = scr.tile([128, 512], F32, tag="osb")
                        nc.vector.tensor_copy(osb[:], ops[dcl][qi][:])
                        nc.sync.dma_start(
                            out_d[qi * 128:(qi + 1) * 128,
                                  dc * 512:(dc + 1) * 512],
                            osb[:])

    if split:
        _split_ctrl_multiwaits(nc)
    return nc


def _prep_inputs(x, q_w, kv_w, o_w, qnorm_scale, knorm_scale, segment_pos,
                 attn_mask):
    """Host-side shard + layout prep. Returns list of 8 input dicts."""
    bf = ml_dtypes.bfloat16
    x = np.asarray(x, np.float32)
    q_w = np.asarray(q_w, np.float32)
    kv_w = np.asarray(kv_w, np.float32)
    o_w = np.asarray(o_w, np.float32)
    qnorm_scale = np.asarray(qnorm_scale, np.float32)
    knorm_scale = np.asarray(knorm_scale, np.float32)
    segment_pos = np.asarray(segment_pos, np.int64)

    # shared (same array object across cores -> no copy)
    wq = np.ascontiguousarray(q_w[:, :, _ORIG]).astype(bf)
    wk = np.ascontiguousarray(kv_w[0][:, :, _ORIG]).astype(bf)
    # wv in [d-tile, d-part, (kh, h)] layout for the stationary-x projection
    wv = np.ascontiguousarray(
        kv_w[1].transpose(1, 0, 2).reshape(ND, 128, K * H)).astype(bf)
    wo = o_w.astype(bf)
    gq = ((1.0 + qnorm_scale[_ORIG]) * SCALE).astype(np.float64)   # [128]
    gk = (1.0 + knorm_scale[_ORIG]).astype(np.float64)
    timescale = ROPE_BASE ** (2.0 * _FREQ.astype(np.float64) / H)  # [128]
    idb = np.eye(128, dtype=bf)

    # boundary-tile triangle masks in [s_p, t] orientation, duplicated per g:
    # r=0 (window lower edge): valid iff sp > tp; r=NWIN-1 (causal): sp <= tp
    sp = np.arange(128)[:, None]
    tp = np.arange(128)[None, :]
    tri0 = (sp > tp).astype(np.float32)
    tri8 = (sp <= tp).astype(np.float32)
    tri = np.concatenate([tri0, tri0, tri8, tri8], axis=1).astype(bf)

    in_maps = []
    for c in range(NCORES):
        b, j = divmod(c, NQT)
        qs = TQ * j
        kvs = qs - WINDOW

        # x^T for local kv window, zero-padded on the left
        xt = np.zeros((D, TKV), bf)
        lo = max(kvs, 0)
        xt[:, lo - kvs:] = x[b, lo:qs + TQ, :].T.astype(bf)

        # rope tables in permuted row order; positions from segment_pos;
        # norm scales folded in (partner-permuted on the sin side)
        pos = np.zeros(TKV, np.float64)
        pos[lo - kvs:] = segment_pos[b, lo:qs + TQ].astype(np.float64)
        theta = pos[None, :] / timescale[:, None]          # [128, TKV]
        cosb = np.cos(theta)
        sinb = np.sin(theta) * _SIGN[:, None].astype(np.float64)
        ckk = (cosb * gk[:, None]).astype(bf)
        skk = (sinb * gk[_PARTNER][:, None]).astype(bf)
        ckq = (cosb[:, WINDOW:] * gq[:, None]).astype(bf)
        skq = (sinb[:, WINDOW:] * gq[_PARTNER][:, None]).astype(bf)

        # den correction: fully-invalid (zero-padded) s-tiles contribute
        # exp(0)=1 per element; the r=0 tile additionally passes its triangle
        nti = max(0, -kvs) // 128
        corr = np.zeros((128, NQT), np.float32)
        for qi in range(NQT):
            nmid = min(7, max(0, nti - qi - 1))
            corr[:, qi] = 128.0 * nmid
            if qi < nti:
                corr[:, qi] += 127.0 - np.arange(128)
        in_maps.append(dict(
            xt=xt, wq=wq, wk=wk, wv=wv, wo=wo,
            ckq=np.ascontiguousarray(ckq), skq=np.ascontiguousarray(skq),
            ckk=np.ascontiguousarray(ckk), skk=np.ascontiguousarray(skk),
            tri=tri, corr=corr, idb=idb))
    return in_maps


def kernel(x, q_w, kv_w, o_w, qnorm_scale, knorm_scale, segment_pos,
           attn_mask, _trace=False):
    import os
    if "nc" not in _module_cache:
        _module_cache["nc"] = _build_module()
    nc = _module_cache["nc"]

    in_maps = _prep_inputs(x, q_w, kv_w, o_w, qnorm_scale, knorm_scale,
                           segment_pos, attn_mask)
    res = run_bass_kernel_spmd(nc, in_maps, core_ids=list(range(NCORES)),
                               trace=_trace,
                               trace_cores=list(range(NCORES)) if _trace
                               else None)
    _module_cache["last_results"] = res

    out = np.zeros((B, T, D), np.float32)
    for c in range(NCORES):
        b, j = divmod(c, NQT)
        out[b, TQ * j:TQ * (j + 1), :] = res.results[c]["out"]
    return out


# revision 31
# speedup vs baseline: 1.1825x; 1.1825x over previous
"""Sliding-window GQA attention (Gemma-style) on 8 Trainium2 NeuronCores.

Sharding: data-parallel over tokens. B=2, T=2048 -> 4096 tokens -> 512
queries per core (core c = 4*b + j handles batch b, queries [512j, 512j+512)).
Each core recomputes k/v for its fixed local window of 1536 tokens
[qs-1024, qs+512) (zero-padded on the left at sequence start), so all 8 cores
run one identical NEFF; per-core differences live entirely in the input data
(sliced x, RoPE tables, den-correction table).

On-chip dataflow (per core):
  phase 1: q/k/v projections with W stationary and x^T moving -> q^T/k^T/v^T
           [H=128 partitions, tokens]; RMSNorm via ones-matmul column sums +
           on1-matmul broadcast of 1/std; norm scales (and SCALE) folded into
           the RoPE tables host-side; 1/std_k folded INTO kTn so attention
           logits need no per-s rescale. RoPE runs in bf16 (head-dim permuted
           on host so rotate-half is a quadrant-local stream_shuffle). v^T
           transposed back to [s, h] via PE transposes.
  phase 2: logits = kTn-tile.T @ qTn-pair (both heads of a kv group, N=256)
           into paired psum; exp on ACT (wide, scale-free since k is
           pre-normalized); boundary s-tiles masked by constant triangles;
           sliding-window validity handled by a host den-correction table
           (invalid tiles contribute exp(0)=1, subtracted from the den).
           PV with ones-column vsb gives enc and den in one accumulation.
  phase 3: output projection accumulating over heads.
"""

import numpy as np
import ml_dtypes

import concourse.bass as bass
import concourse.mybir as mybir
import concourse.tile as tile
from concourse import library_config
from concourse.masks import make_identity
from concourse.bass_utils import run_bass_kernel_spmd

AF = mybir.ActivationFunctionType
ALU = mybir.AluOpType
F32 = mybir.dt.float32
BF16 = mybir.dt.bfloat16

B, T, D = 2, 2048, 2048
N, K, H = 16, 8, 128
G = N // K
SOFT_CAP = 50.0
WINDOW = 1024
SCALE = H ** -0.5
ROPE_BASE = 10000.0
EPS = 1e-6

TQ = 512            # queries per core
TKV = 1536          # kv window per core
VST = 129           # per-s-tile width in vsb: 128 v cols + ones column
NQT = TQ // 128     # 4 q-tiles
NST = TKV // 128    # 12 s-tiles
ND = D // 128       # 16 d-tiles
NWIN = 9            # s-tiles in a q-tile's window
NCORES = 8
EVW = 132           # per-g stride in the ev psum tile (16B aligned)

# quadrant-local half swap for stream_shuffle (32-partition groups)
SWAP16 = list(range(16, 32)) + list(range(16))


def _rope_perm():
    """orig[p] = original head-dim index stored at partition p; freq[p];
    sign[p] for the sin table."""
    orig = np.zeros(128, np.int64)
    freq = np.zeros(128, np.int64)
    sign = np.zeros(128, np.float32)
    for p in range(128):
        qd, o = divmod(p, 32)
        if o < 16:
            orig[p] = 16 * qd + o
            freq[p] = 16 * qd + o
            sign[p] = -1.0
        else:
            orig[p] = 64 + 16 * qd + (o - 16)
            freq[p] = 16 * qd + (o - 16)
            sign[p] = 1.0
    return orig, freq, sign


_ORIG, _FREQ, _SIGN = _rope_perm()
# partner index under SWAP16 (the other half of each rotate pair)
_PARTNER = np.array([32 * (p // 32) + (p % 32 + 16) % 32 for p in range(128)])

_module_cache = {}

# Instruction types lowered to CTRL encodings: the walrus build in this
# container supports only ONE sync-wait on them ("Too many sync wait
# commands" / "ISA wrong length" in codegen otherwise).
_CTRL_TYPES = ("InstDrain", "InstNoOp", "InstISA", "InstEventSemaphore")


def _split_ctrl_multiwaits(nc, maxw=1):
    """Move excess sem-waits off CTRL-type instructions onto preceding
    same-engine NoOps (same engine queue => identical ordering semantics)."""
    import concourse.mybir as mybir
    for f in nc.m.functions:
        for blk in f.blocks:
            insts = blk.instructions
            out = []
            changed = False
            for inst in insts:
                si = inst.sync_info
                if (si is not None and si.on_wait
                        and len(si.on_wait) > maxw):
                    waits = list(si.on_wait)
                    extra, keep = waits[:-maxw], waits[-maxw:]
                    for k, w in enumerate(extra):
                        nop = mybir.InstNoOp(name=f"{inst.name}-ws{k}",
                                             ins=[], outs=[])
                        nop.engine = inst.engine
                        nop.sync_info = mybir.SyncInfo(on_wait=[w],
                                                       on_update=[])
                        out.append(nop)
                    si.on_wait = keep
                    changed = True
                out.append(inst)
            if changed:
                insts[:] = out


def _build_module(split=True):
    nc = bass.Bass("TRN2", target_bir_lowering=False, debug=False)

    xt_d = nc.dram_tensor("xt", (D, TKV), BF16, kind="ExternalInput").ap()
    wq_d = nc.dram_tensor("wq", (N, D, H), BF16, kind="ExternalInput").ap()
    wk_d = nc.dram_tensor("wk", (K, D, H), BF16, kind="ExternalInput").ap()
    wv_d = nc.dram_tensor("wv", (ND, 128, K * H), BF16,
                          kind="ExternalInput").ap()
    wo_d = nc.dram_tensor("wo", (N, H, D), BF16, kind="ExternalInput").ap()
    ckq_d = nc.dram_tensor("ckq", (H, TQ), BF16, kind="ExternalInput").ap()
    skq_d = nc.dram_tensor("skq", (H, TQ), BF16, kind="ExternalInput").ap()
    ckk_d = nc.dram_tensor("ckk", (H, TKV), BF16, kind="ExternalInput").ap()
    skk_d = nc.dram_tensor("skk", (H, TKV), BF16, kind="ExternalInput").ap()
    tri_d = nc.dram_tensor("tri", (128, 512), BF16, kind="ExternalInput").ap()
    corr_d = nc.dram_tensor("corr", (128, NQT), F32,
                            kind="ExternalInput").ap()
    idb_d = nc.dram_tensor("idb", (128, 128), BF16, kind="ExternalInput").ap()
    out_d = nc.dram_tensor("out", (TQ, D), F32, kind="ExternalOutput").ap()

    with tile.TileContext(nc) as tc:
        with tc.tile_pool(name="const", bufs=1) as cst, \
             tc.tile_pool(name="acc", bufs=1) as acc, \
             tc.tile_pool(name="wst", bufs=2) as wst, \
             tc.tile_pool(name="wvp", bufs=4) as wvp, \
             tc.tile_pool(name="wost", bufs=8) as wost, \
             tc.tile_pool(name="scr", bufs=2) as scr, \
             tc.tile_pool(name="pp", bufs=3) as pp, \
             tc.tile_pool(name="psA", bufs=4, space="PSUM") as psA, \
             tc.tile_pool(name="psB", bufs=4, space="PSUM") as psB:

            # ---- constants / preloads ----
            # first two q-heads' weights load before the big xts transfer so
            # the first projection matmuls start as soon as x^T tiles land
            w_pre = {}
            for n0 in range(2):
                wp = wst.tile([128, ND * H], BF16, tag="w", name=f"w_pre{n0}")
                nc.sync.dma_start(
                    wp[:].rearrange("p (d h) -> p d h", d=ND),
                    wq_d[n0].rearrange("(d p) h -> p d h", p=128))
                w_pre[n0] = wp
            xts = cst.tile([128, ND * TKV], BF16, tag="xts")
            xt_r = xt_d.rearrange("(d p) t -> d p t", p=128)
            for d in range(ND):
                nc.sync.dma_start(xts[:, d * TKV:(d + 1) * TKV], xt_r[d])
            ckq_t = cst.tile([H, TQ], BF16, tag="ckq")
            nc.sync.dma_start(ckq_t[:], ckq_d[:])
            skq_t = cst.tile([H, TQ], BF16, tag="skq")
            nc.sync.dma_start(skq_t[:], skq_d[:])
            ckk_t = cst.tile([H, TKV], BF16, tag="ckk")
            nc.sync.dma_start(ckk_t[:], ckk_d[:])
            skk_t = cst.tile([H, TKV], BF16, tag="skk")
            nc.sync.dma_start(skk_t[:], skk_d[:])
            tri_t = cst.tile([128, 512], BF16, tag="tri")
            nc.sync.dma_start(tri_t[:], tri_d[:])
            corr_t = cst.tile([128, NQT], F32, tag="corr")
            nc.sync.dma_start(corr_t[:], corr_d[:])
            idb_t = cst.tile([128, 128], BF16, tag="idb")
            nc.sync.dma_start(idb_t[:], idb_d[:])
            ones_bf = cst.tile([128, 1], BF16, tag="ones")
            nc.vector.memset(ones_bf[:], 1.0)
            on1 = cst.tile([1, 128], F32, tag="on1")
            nc.vector.memset(on1[:], 1.0)
            eps_t = cst.tile([1, 1], F32, tag="eps")
            nc.vector.memset(eps_t[:], EPS)

            # ---- big accumulators ----
            qTn = acc.tile([128, N * TQ], BF16, tag="qTn")
            kTn = acc.tile([128, K * TKV], BF16, tag="kTn")
            vsb = acc.tile([128, K * NST * VST], BF16, tag="vsb")
            nc.vector.memset(vsb[:], 1.0)
            encT = acc.tile([128, N * NQT * 128], BF16, tag="encT")

            def rope(src_bf, cka, ska, out_slice, strided=False):
                rot = scr.tile([128, 512], BF16, tag="rot")
                nc.vector.stream_shuffle(rot[:], src_bf[:], SWAP16)
                t1 = scr.tile([128, 512], BF16, tag="t1")
                nc.vector.tensor_mul(t1[:], src_bf[:], cka)
                t2 = scr.tile([128, 512], BF16, tag="t2")
                nc.gpsimd.tensor_mul(t2[:], rot[:], ska)
                if strided:
                    nc.vector.tensor_add(
                        out_slice,
                        t1[:].rearrange("p (a b) -> p a b", a=NQT),
                        t2[:].rearrange("p (a b) -> p a b", a=NQT))
                else:
                    nc.vector.tensor_add(out_slice, t1[:], t2[:])

            # ---- phase 1a: q heads ----
            q_s1 = q_s2 = None
            for n in range(N + 2):
                nstate = None
                if n < N:
                    if n in w_pre:
                        w_t = w_pre[n]
                    else:
                        w_t = wst.tile([128, ND * H], BF16, tag="w")
                        nc.sync.dma_start(
                            w_t[:].rearrange("p (d h) -> p d h", d=ND),
                            wq_d[n].rearrange("(d p) h -> p d h", p=128))
                    ps = psA.tile([128, 512], F32, tag="big")
                    for d in range(ND):
                        nc.tensor.matmul(
                            ps[:], w_t[:, d * H:(d + 1) * H],
                            xts[:, d * TKV + 1024:d * TKV + 1536],
                            start=(d == 0), stop=(d == ND - 1))
                    sq = scr.tile([128, 512], BF16, tag="sq")
                    nc.scalar.activation(sq[:], ps[:], AF.Square)
                    nstate = (n, ps, sq)
                if q_s1 is not None:
                    n1, ps1, sq1 = q_s1
                    ssp = psB.tile([1, 512], F32, tag="sm")
                    nc.tensor.matmul(ssp[:], ones_bf[:], sq1[:],
                                     start=True, stop=True)
                    lns = scr.tile([1, 512], F32, tag="row")
                    nc.scalar.activation(lns[:], ssp[:], AF.Ln,
                                         scale=1.0 / H, bias=eps_t[:])
                    rst = scr.tile([1, 512], F32, tag="row2")
                    nc.scalar.activation(rst[:], lns[:], AF.Exp, scale=-0.5)
                    q_s1 = (n1, ps1, rst)
                if q_s2 is not None:
                    n2, ps2, rst2 = q_s2
                    rbp = psB.tile([128, 512], F32, tag="sm")
                    nc.tensor.matmul(rbp[:], on1[:], rst2[:],
                                     start=True, stop=True)
                    rstb = scr.tile([128, 512], BF16, tag="rstb")
                    nc.vector.tensor_copy(rstb[:], rbp[:])
                    qn = scr.tile([128, 512], BF16, tag="qn")
                    nc.vector.tensor_mul(qn[:], ps2[:], rstb[:])
                    # qTn layout: [128, (kh, qi, g, t)] so the two q-heads of
                    # a kv group sit adjacent per q-tile (256-wide logits rhs)
                    kh2, g2 = divmod(n2, G)
                    qfull = qTn[:]
                    q_ap = bass.AP(
                        qfull.tensor,
                        qfull.offset + (kh2 * G * NQT + g2) * 128,
                        [qfull.ap[0], [G * 128, NQT], [1, 128]])
                    rope(qn, ckq_t[:, 0:512], skq_t[:, 0:512], q_ap,
                         strided=True)
                q_s2 = q_s1
                q_s1 = nstate

            # ---- phase 2: attention (interleaved with the k projection) ----
            a_s1 = a_s2 = None
            iters = [(kh, qi) for kh in range(K) for qi in range(NQT)]
            a_it = [0]
            # psum pairing for wide exp: (r0,r8) share a tile (both get the
            # triangle mask), then (1,2),(3,4),(5,6),(7,)
            PAIRS = [(0, 8), (1, 2), (3, 4), (5, 6), (7,)]

            def attn_step(it):
                nstate = None
                if it < len(iters):
                        kh, qi = iters[it]
                        probs = pp.tile([128, NWIN * 256], BF16, tag="probs")
                        eeT = scr.tile([128, 512], BF16, tag="eeT")
                        for pr in PAIRS:
                            lgp = psB.tile([128, 256 * len(pr)], F32,
                                           tag="sm")
                            for i, r in enumerate(pr):
                                st = qi + r
                                nc.tensor.matmul(
                                    lgp[:, i * 256:(i + 1) * 256],
                                    kTn[:, kh * TKV + st * 128:
                                        kh * TKV + (st + 1) * 128],
                                    qTn[:, (kh * NQT + qi) * 256:
                                        (kh * NQT + qi + 1) * 256],
                                    start=True, stop=True)
                            if pr[0] == 0:
                                out = eeT[:]
                            else:
                                out = probs[:, pr[0] * 256:
                                            (pr[-1] + 1) * 256]
                            nc.scalar.activation(out, lgp[:], AF.Exp)
                        # boundary triangle masks for r=0 and r=8
                        pfull = probs[:]
                        p_ap = bass.AP(
                            pfull.tensor, pfull.offset,
                            [pfull.ap[0], [(NWIN - 1) * 256, 2], [1, 256]])
                        nc.vector.tensor_tensor(
                            p_ap,
                            eeT[:].rearrange("p (a b) -> p a b", a=2),
                            tri_t[:].rearrange("p (a b) -> p a b", a=2),
                            op=ALU.mult)
                        nstate = (kh, qi, probs)
                if a_s2 is not None:
                        kh0, qi0, probs0 = a_s2
                        ev = psB.tile([128, 2 * EVW], F32, tag="sm")
                        for g in range(G):
                            for r in range(NWIN):
                                st = qi0 + r
                                off = (kh0 * NST + st) * VST
                                nc.tensor.matmul(
                                    ev[:, g * EVW:g * EVW + VST],
                                    probs0[:, r * 256 + g * 128:
                                           r * 256 + (g + 1) * 128],
                                    vsb[:, off:off + VST],
                                    start=(r == 0), stop=(r == NWIN - 1))
                        evf = ev[:]
                        den_ap = bass.AP(evf.tensor, evf.offset + 128,
                                         [evf.ap[0], [EVW, 2]])
                        den2 = scr.tile([128, 2], F32, tag="den2")
                        nc.vector.tensor_scalar(
                            den2[:], den_ap, corr_t[:, qi0:qi0 + 1], None,
                            op0=ALU.subtract)
                        rden2 = scr.tile([128, 2], F32, tag="rden2")
                        nc.vector.reciprocal(rden2[:], den2[:])
                        v_ap = bass.AP(evf.tensor, evf.offset,
                                       [evf.ap[0], [EVW, 2], [1, 128]])
                        rdf = rden2[:]
                        rd_ap = bass.AP(rdf.tensor, rdf.offset,
                                        [rdf.ap[0], [1, 2], [0, 128]])
                        enc_sb = scr.tile([128, 256], BF16, tag="encsb")
                        nc.vector.tensor_tensor(
                            enc_sb[:].rearrange("p (a b) -> p a b", a=2),
                            v_ap, rd_ap, op=ALU.mult)
                        etp = psB.tile([128, 256], BF16, tag="sm")
                        for g in range(G):
                            nc.tensor.matmul(
                                etp[:, g * 128:(g + 1) * 128],
                                enc_sb[:, g * 128:(g + 1) * 128], idb_t[:],
                                is_transpose=True, start=True, stop=True)
                        efull = encT[:]
                        e_ap = bass.AP(
                            efull.tensor,
                            efull.offset + (kh0 * G * NQT + qi0) * 128,
                            [efull.ap[0], [NQT * 128, 2], [1, 128]])
                        nc.vector.tensor_copy(
                            e_ap, etp[:].rearrange("p (a b) -> p a b", a=2))
                return nstate

            # ---- phase 1c: v projection, directly in [s, h] orientation ----
            # stationary = x^T d-tile [128 d, 128 s]; moving = wv for 4 kv
            # heads [128 d, 512]; 3 rounds x 4 s-tiles x 2 head-halves use
            # all 8 psum banks; wv streams per d-tile (re-read each round).
            for c in range(3):
                vps = []
                for stq in range(4):
                    pa = psA.tile([128, 512], F32, tag="big",
                                  name=f"vA_{c}_{stq}")
                    pb = psB.tile([128, 512], F32, tag="sm",
                                  name=f"vB_{c}_{stq}")
                    vps.append((pa, pb))
                for d in range(ND):
                    wvt = wvp.tile([128, K * H], BF16, tag="wv")
                    nc.sync.dma_start(wvt[:], wv_d[d])
                    for stq in range(4):
                        pa, pb = vps[stq]
                        xsl = xts[:, d * TKV + c * 512 + stq * 128:
                                  d * TKV + c * 512 + (stq + 1) * 128]
                        nc.tensor.matmul(pa[:], xsl, wvt[:, 0:512],
                                         start=(d == 0), stop=(d == ND - 1))
                        nc.tensor.matmul(pb[:], xsl, wvt[:, 512:1024],
                                         start=(d == 0), stop=(d == ND - 1))
                for stq in range(4):
                    st = c * 4 + stq
                    pa, pb = vps[stq]
                    vf = vsb[:]
                    for half, pt in ((0, pa), (1, pb)):
                        v_ap = bass.AP(
                            vf.tensor,
                            vf.offset + (half * 4 * NST + st) * VST,
                            [vf.ap[0], [NST * VST, 4], [1, 128]])
                        nc.vector.tensor_copy(
                            v_ap, pt[:].rearrange("p (a b) -> p a b", a=4))

            def attn_advance(k_steps):
                nonlocal a_s1, a_s2
                for _ in range(k_steps):
                    if a_it[0] >= len(iters) + 2:
                        return
                    ns = attn_step(a_it[0])
                    a_it[0] += 1
                    a_s2 = a_s1
                    a_s1 = ns

            # ---- phase 1b: k heads (1/std_k folded into kTn), attention
            # iters interleaved per completed kv head so the two MM streams
            # fill each other's dependency bubbles ----
            k_s1 = k_s2 = None
            nchunks = K * 3
            for ci in range(nchunks + 2):
                nstate = None
                if ci < nchunks:
                    kh, c = divmod(ci, 3)
                    if c == 0:
                        w_t = wst.tile([128, ND * H], BF16, tag="w")
                        nc.sync.dma_start(
                            w_t[:].rearrange("p (d h) -> p d h", d=ND),
                            wk_d[kh].rearrange("(d p) h -> p d h", p=128))
                    ps = psA.tile([128, 512], F32, tag="big")
                    for d in range(ND):
                        nc.tensor.matmul(
                            ps[:], w_t[:, d * H:(d + 1) * H],
                            xts[:, d * TKV + c * 512:d * TKV + (c + 1) * 512],
                            start=(d == 0), stop=(d == ND - 1))
                    sq = scr.tile([128, 512], BF16, tag="sq")
                    nc.scalar.activation(sq[:], ps[:], AF.Square)
                    nstate = (kh, c, ps, sq)
                if k_s1 is not None:
                    kh1, c1, ps1, sq1 = k_s1
                    ssp = psB.tile([1, 512], F32, tag="sm")
                    nc.tensor.matmul(ssp[:], ones_bf[:], sq1[:],
                                     start=True, stop=True)
                    lns = scr.tile([1, 512], F32, tag="row")
                    nc.scalar.activation(lns[:], ssp[:], AF.Ln,
                                         scale=1.0 / H, bias=eps_t[:])
                    rks = scr.tile([1, 512], F32, tag="row2")
                    nc.scalar.activation(rks[:], lns[:], AF.Exp, scale=-0.5)
                    k_s1 = (kh1, c1, ps1, rks)
                if k_s2 is not None:
                    kh2, c2, ps2, rks2 = k_s2
                    rbp = psB.tile([128, 512], F32, tag="sm")
                    nc.tensor.matmul(rbp[:], on1[:], rks2[:],
                                     start=True, stop=True)
                    rkb = scr.tile([128, 512], BF16, tag="rstb")
                    nc.vector.tensor_copy(rkb[:], rbp[:])
                    kn = scr.tile([128, 512], BF16, tag="qn")
                    nc.vector.tensor_mul(kn[:], ps2[:], rkb[:])
                    rope(kn, ckk_t[:, c2 * 512:(c2 + 1) * 512],
                         skk_t[:, c2 * 512:(c2 + 1) * 512],
                         kTn[:, kh2 * TKV + c2 * 512:
                             kh2 * TKV + (c2 + 1) * 512])
                    # release attention iters at ~4/3 per chunk (lag 2) so
                    # the k and attention MM streams stay finely interleaved;
                    # never past the last completed kv head
                    ci2 = 3 * kh2 + c2
                    avail = NQT * ((ci2 + 1) // 3)
                    target = min(avail, max(0, (NQT * (ci2 + 1)) // 3 - 2))
                    attn_advance(target - a_it[0])
                k_s2 = k_s1
                k_s1 = nstate
            attn_advance(len(iters) + 2 - a_it[0])

            # ---- phase 3: output projection ----
            for dc in range(4):
                ops = [psA.tile([128, 512], F32, tag="big", name=f"op_{dc}_{qi}")
                       for qi in range(NQT)]
                for n in range(N):
                    wo_sl = wost.tile([128, 512], BF16, tag="wo")
                    nc.sync.dma_start(wo_sl[:],
                                      wo_d[n][:, dc * 512:(dc + 1) * 512])
                    for qi in range(NQT):
                        nc.tensor.matmul(
                            ops[qi][:],
                            encT[:, (n * NQT + qi) * 128:(n * NQT + qi + 1) * 128],
                            wo_sl[:], start=(n == 0), stop=(n == N - 1))
                for qi in range(NQT):
                    osb = scr.tile([128, 512], F32, tag="osb")
                    nc.vector.tensor_copy(osb[:], ops[qi][:])
                    nc.sync.dma_start(
                        out_d[qi * 128:(qi + 1) * 128, dc * 512:(dc + 1) * 512],
                        osb[:])

    if split:
        _split_ctrl_multiwaits(nc)
    return nc


def _prep_inputs(x, q_w, kv_w, o_w, qnorm_scale, knorm_scale, segment_pos,
                 attn_mask):
    """Host-side shard + layout prep. Returns list of 8 input dicts."""
    bf = ml_dtypes.bfloat16
    x = np.asarray(x, np.float32)
    q_w = np.asarray(q_w, np.float32)
    kv_w = np.asarray(kv_w, np.float32)
    o_w = np.asarray(o_w, np.float32)
    qnorm_scale = np.asarray(qnorm_scale, np.float32)
    knorm_scale = np.asarray(knorm_scale, np.float32)
    segment_pos = np.asarray(segment_pos, np.int64)

    # shared (same array object across cores -> no copy)
    wq = np.ascontiguousarray(q_w[:, :, _ORIG]).astype(bf)
    wk = np.ascontiguousarray(kv_w[0][:, :, _ORIG]).astype(bf)
    # wv in [d-tile, d-part, (kh, h)] layout for the stationary-x projection
    wv = np.ascontiguousarray(
        kv_w[1].transpose(1, 0, 2).reshape(ND, 128, K * H)).astype(bf)
    wo = o_w.astype(bf)
    gq = ((1.0 + qnorm_scale[_ORIG]) * SCALE).astype(np.float64)   # [128]
    gk = (1.0 + knorm_scale[_ORIG]).astype(np.float64)
    timescale = ROPE_BASE ** (2.0 * _FREQ.astype(np.float64) / H)  # [128]
    idb = np.eye(128, dtype=bf)

    # boundary-tile triangle masks in [s_p, t] orientation, duplicated per g:
    # r=0 (window lower edge): valid iff sp > tp; r=NWIN-1 (causal): sp <= tp
    sp = np.arange(128)[:, None]
    tp = np.arange(128)[None, :]
    tri0 = (sp > tp).astype(np.float32)
    tri8 = (sp <= tp).astype(np.float32)
    tri = np.concatenate([tri0, tri0, tri8, tri8], axis=1).astype(bf)

    in_maps = []
    for c in range(NCORES):
        b, j = divmod(c, NQT)
        qs = TQ * j
        kvs = qs - WINDOW

        # x^T for local kv window, zero-padded on the left
        xt = np.zeros((D, TKV), bf)
        lo = max(kvs, 0)
        xt[:, lo - kvs:] = x[b, lo:qs + TQ, :].T.astype(bf)

        # rope tables in permuted row order; positions from segment_pos;
        # norm scales folded in (partner-permuted on the sin side)
        pos = np.zeros(TKV, np.float64)
        pos[lo - kvs:] = segment_pos[b, lo:qs + TQ].astype(np.float64)
        theta = pos[None, :] / timescale[:, None]          # [128, TKV]
        cosb = np.cos(theta)
        sinb = np.sin(theta) * _SIGN[:, None].astype(np.float64)
        ckk = (cosb * gk[:, None]).astype(bf)
        skk = (sinb * gk[_PARTNER][:, None]).astype(bf)
        ckq = (cosb[:, WINDOW:] * gq[:, None]).astype(bf)
        skq = (sinb[:, WINDOW:] * gq[_PARTNER][:, None]).astype(bf)

        # den correction: fully-invalid (zero-padded) s-tiles contribute
        # exp(0)=1 per element; the r=0 tile additionally passes its triangle
        nti = max(0, -kvs) // 128
        corr = np.zeros((128, NQT), np.float32)
        for qi in range(NQT):
            nmid = min(7, max(0, nti - qi - 1))
            corr[:, qi] = 128.0 * nmid
            if qi < nti:
                corr[:, qi] += 127.0 - np.arange(128)
        in_maps.append(dict(
            xt=xt, wq=wq, wk=wk, wv=wv, wo=wo,
            ckq=np.ascontiguousarray(ckq), skq=np.ascontiguousarray(skq),
            ckk=np.ascontiguousarray(ckk), skk=np.ascontiguousarray(skk),
            tri=tri, corr=corr, idb=idb))
    return in_maps


def kernel(x, q_w, kv_w, o_w, qnorm_scale, knorm_scale, segment_pos,
           attn_mask, _trace=False):
    import os
    if "nc" not in _module_cache:
        _module_cache["nc"] = _build_module()
    nc = _module_cache["nc"]

    in_maps = _prep_inputs(x, q_w, kv_w, o_w, qnorm_scale, knorm_scale,
                           segment_pos, attn_mask)
    res = run_bass_kernel_spmd(nc, in_maps, core_ids=list(range(NCORES)),
                               trace=_trace,
                               trace_cores=list(range(NCORES)) if _trace
                               else None)
    _module_cache["last_results"] = res

    out = np.zeros((B, T, D), np.float32)
    for c in range(NCORES):
        b, j = divmod(c, NQT)
        out[b, TQ * j:TQ * (j + 1), :] = res.results[c]["out"]
    return out
